# revision 25
# baseline (speedup 1.0000x reference)
"""Multi-head attention (B=8, N=1024, C=768, H=12) on 8 TRN2 NeuronCores.

Strategy: pure data parallelism over the batch dim — each core computes one
batch element's full attention block. Weights replicated; no collectives.

v3 (fp16 operands, LDWEIGHTS amortization, kt-granular pipeline):
 - HW measurement: a bf16/fp16 N=512 matmul with its own stationary operand
   costs ~296 ns, but two matmuls SHARING one LDWEIGHTS cost ~156 ns each
   (the moving operand streams ~2 cols/cycle at 16-bit; the per-matmul
   weight load is NOT hidden).  Every matmul group in v3 therefore streams
   two+ moving blocks per stationary load:
     * qk features: per (m, c): one LDW, two 512-token matmuls (q halves)
     * v features / proj: per (t|c): one LDW, 512+256 column matmuls
     * scores: one N=1024 matmul per (kt, head) — fp16 PSUM output fits
       1024 columns in one 2 KiB bank; the two heads of a pair run as
       concurrent row-tiled matmuls (contraction rows 0:64 / 64:128)
     * AV: per (kt, head): one LDW of the v pair-block, two 512-col
       matmuls accumulating the q2=0 / q2=1 psum tiles
 - x is loaded TRANSPOSED by the DMA XBAR (dma_start_transpose, 16-bit):
   xT[p, c, t] = x[t, c*128+p]; no PE transposes at all.
 - All inputs host-cast to fp16 (better mantissa than bf16; ranges are
   tiny).  Scores live in fp16 PSUM (|s| < 100 << 65504).
 - Pipeline: per head pair j, the kt loop emits scores(j, kt) interleaved
   with AV(j-1, kt-2) plus one queued feature/projection filler group per
   slot; Exp on ACT (~2.1 us per kt) is the pacing engine.
 - AV rides softmax denominators in the M=128 matmul via ones columns of
   the v pair-blocks; normalize = 4 reciprocals (rows 32/64) + selector
   matmul broadcast + staged multiply.

v pair-block layout: per head pair j the columns are
  [ vA(0:64) | onesA(64) | onesB(65) | zeros(66:97) | vB(97:161) ]
lhsT_A = block[0:128]   -> psum rows: 0-63 A-out, 64 A-sums
lhsT_B = block[33:161]  -> psum rows: 32 B-sums, 64-127 B-out

Timing methodology (test.py): the body is wrapped in a hardware For_i
loop; per-iteration time = (wall(hi) - wall(lo)) / (hi - lo), which
cancels the ~2s axon-tunnel call overhead.
"""

import os
import numpy as np

import concourse.bass as bass
import concourse.tile as tile
from concourse import bacc, mybir
from concourse.bass_utils import run_bass_kernel_spmd

B, N, C, H, HD = 8, 1024, 768, 12, 64
C3 = 3 * C
P = 128
NT = N // P   # 8 token tiles
CK = C // P   # 6 C chunks
QC = 512
NQ = N // QC  # 2
NJ = H // 2   # 6 head pairs
f32 = mybir.dt.float32
f16 = mybir.dt.float16

PW = 161       # v pair block width
OFS_B = 33     # lhsT_B offset within the block
VB_OFS = 97    # vB column offset

MODE = os.environ.get("ATTN_MM_MODE", "f16")
USE_XBAR = os.environ.get("ATTN_XBAR", "0") == "1"


class Ctx:
    def __init__(self, **kw):
        self.__dict__.update(kw)


def _qk_feat2(cx, m):
    """q/k feature chunk m for BOTH q halves (shared LDW per c-chunk)."""
    nc = cx.nc
    ps = cx.fp.tile([P, 2, QC], f32, tag="fp")
    for c in range(CK):
        for q2 in range(NQ):
            nc.tensor.matmul(
                ps[:, q2],
                lhsT=cx.wq_s[:, c, m * P : (m + 1) * P],
                rhs=cx.xT_s[:, c, q2 * QC : (q2 + 1) * QC],
                start=(c == 0),
                stop=(c == CK - 1),
            )
    nc.vector.tensor_scalar_add(
        out=cx.qkT_s[:, m, :],
        in0=ps.rearrange("p a b -> p (a b)"),
        scalar1=cx.qkvb_qk[:, m : m + 1],
    )


def _v_feat(cx, t):
    """v features for token tile t (one LDW per c, 512+256 col matmuls)."""
    nc = cx.nc
    ps = cx.fp.tile([P, 2, QC], f32, tag="fp")
    for c in range(CK):
        nc.tensor.matmul(
            ps[:, 0],
            lhsT=cx.xT_s[:, c, t * P : (t + 1) * P],
            rhs=cx.wq_s[:, c, 2 * C : 2 * C + QC],
            start=(c == 0), stop=(c == CK - 1),
        )
        nc.tensor.matmul(
            ps[:, 1, 0 : C - QC],
            lhsT=cx.xT_s[:, c, t * P : (t + 1) * P],
            rhs=cx.wq_s[:, c, 2 * C + QC : 3 * C],
            start=(c == 0), stop=(c == CK - 1),
        )
    for nv in range(2):
        nsz = min(QC, C - nv * QC)
        h0, nh = nv * 8, nsz // HD
        pv = ps[:, nv, :nsz].rearrange("p (h j) -> p h j", j=HD)
        j0 = h0 // 2
        nc.vector.tensor_add(
            out=cx.vnat_w[:, t, j0 : j0 + nh // 2, 0:HD],
            in0=pv[:, 0::2],
            in1=cx.vb_bc[:, h0 : h0 + nh : 2, :],
        )
        nc.vector.tensor_add(
            out=cx.vnat_w[:, t, j0 : j0 + nh // 2, VB_OFS : VB_OFS + HD],
            in0=pv[:, 1::2],
            in1=cx.vb_bc[:, h0 + 1 : h0 + nh : 2, :],
        )


def _proj_tile(cx, t):
    """output projection for token tile t + store."""
    nc = cx.nc
    ps = cx.fp.tile([P, 2, QC], f32, tag="fp")
    for c in range(CK):
        nc.tensor.matmul(
            ps[:, 0],
            lhsT=cx.concatT_s[:, c, t * P : (t + 1) * P],
            rhs=cx.wp_s[:, c, 0:QC],
            start=(c == 0), stop=(c == CK - 1),
        )
        nc.tensor.matmul(
            ps[:, 1, 0 : C - QC],
            lhsT=cx.concatT_s[:, c, t * P : (t + 1) * P],
            rhs=cx.wp_s[:, c, QC:C],
            start=(c == 0), stop=(c == CK - 1),
        )
    out_t = cx.outs.tile([P, C], f16, tag="ot")
    nc.vector.tensor_add(out=out_t[:, 0:QC], in0=ps[:, 0], in1=cx.pb_bc[:, 0:QC])
    nc.vector.tensor_add(
        out=out_t[:, QC:C], in0=ps[:, 1, 0 : C - QC], in1=cx.pb_bc[:, QC:C]
    )
    [nc.sync, nc.scalar][t % 2].dma_start(cx.out_r[t], out_t)


def _score_kt(cx, j, kt, exp4):
    """scores + exp for head pair j, k-token tile kt, both q halves.

    Per head: one LDWEIGHTS of the kT tile, two N=512 matmuls (q halves).
    All four results land in ONE [P, 4, 512] psum tile (slots: A-q0, A-q1,
    B-q0, B-q1) so a SINGLE Exp call covers 2048 columns — the ACT engine
    has a ~1.1us fixed cost per call and only ~0.32 ns/col marginal cost,
    so halving the call count is a ~35% ACT saving.
    exp4 has layout [P, NT, 4, QC] with the same slot order.
    """
    nc = cx.nc
    Act = mybir.ActivationFunctionType
    ks = slice(kt * P, (kt + 1) * P)
    psk = cx.sc.tile([P, 4, QC], f32, tag="sc", name=f"ps{j}_{kt}")
    for half, rows in ((0, slice(0, HD)), (1, slice(HD, P))):
        for q2 in range(NQ):
            nc.tensor.matmul(
                psk[:, 2 * half + q2],
                lhsT=cx.qkT_s[rows, NJ + j, ks],
                rhs=cx.qkT_s[rows, j, q2 * QC : (q2 + 1) * QC],
                start=True, stop=True,
            )
    nc.scalar.activation(exp4[:, kt], psk, Act.Exp, scale=0.125)


def _av_kt(cx, j, half, kt, exp4, psAV2):
    """AV accumulation for pair j, head `half`, k-tile kt (1 LDW, 2 MMs)."""
    nc = cx.nc
    st, sp = kt == 0, kt == NT - 1
    ofs = 0 if half == 0 else OFS_B
    lhsT = cx.vnat_s[:, kt, j * PW + ofs : j * PW + ofs + P]
    for q2 in range(NQ):
        nc.tensor.matmul(
            psAV2[:, q2],
            lhsT=lhsT,
            rhs=exp4[:, kt, 2 * half + q2],
            start=st, stop=sp,
        )


def _norm_half(cx, j, half, psAV2):
    """normalize pair j, head `half` (A sums row 64, B sums row 32)."""
    nc = cx.nc
    srow = 64 if half == 0 else 32
    rows = slice(0, HD) if half == 0 else slice(HD, P)
    with nc.allow_low_precision(reason="normalization factor in fp16"):
        nc.vector.reciprocal(cx.r_ab[srow : srow + 1, 0], psAV2[srow : srow + 1, 0])
        nc.vector.reciprocal(cx.r_ab[srow : srow + 1, 1], psAV2[srow : srow + 1, 1])
    # sel65 maps r_ab row 64 -> psum rows 0:64 and row 32 -> rows 64:128;
    # only this half's output rows are read below, the rest is stale.
    psR = cx.fp.tile([P, 2, QC], f32, tag="fp", name=f"psR{j}_{half}")
    for q2 in range(NQ):
        nc.tensor.matmul(
            psR[:, q2], lhsT=cx.sel65, rhs=cx.r_ab[:, q2], start=True, stop=True
        )
    rbc = cx.rbcp.tile([P, 2, QC], f16, tag="rbc", name=f"rbc{j}_{half}")
    nc.vector.tensor_copy(rbc[rows], psR[rows])
    for q2 in range(NQ):
        qs = slice(q2 * QC, (q2 + 1) * QC)
        nc.vector.tensor_mul(
            out=cx.concatT_s[rows, j, qs], in0=psAV2[rows, q2], in1=rbc[rows, q2]
        )


def _setup(tc, persist, exps, outs, rbcp, sc, fp, avp, out_d, dbg):
    """One-time tile allocation + constant fills (outside the timing loop)."""
    nc = tc.nc
    cx = Ctx(
        nc=nc, dbg=dbg,
        xT_s=persist.tile([P, CK, N], f16, name="xT_s"),
        wq_s=persist.tile([P, CK, C3], f16, name="wq_s"),
        wp_s=persist.tile([P, CK, C], f16, name="wp_s"),
        qkT_s=persist.tile([P, 2 * CK, N], f16, name="qkT_s"),
        vnat_s=persist.tile([P, NT, NJ * PW], f16, name="vnat_s"),
        concatT_s=persist.tile([P, CK, N], f16, name="concatT_s"),
        qkvb_qk=persist.tile([P, 2 * CK], f32, name="qkvb_qk"),
        vb_bc=persist.tile([P, H, HD], f32, name="vb_bc"),
        pb_bc=persist.tile([P, C], f32, name="pb_bc"),
        r_ab=persist.tile([65, 2, QC], f16, name="r_ab"),
        sel65=persist.tile([65, P], f16, name="sel65"),
        out_r=out_d.rearrange("(t p) c -> t p c", p=P),
        exps=exps, outs=outs, rbcp=rbcp, sc=sc, fp=fp, avp=avp,
    )
    cx.vnat_w = cx.vnat_s.rearrange("p t (j w) -> p t j w", w=PW)

    ones_f = persist.tile([P, 1], f32)
    zero_f = persist.tile([P, 1], f32)
    nc.vector.memset(ones_f, 1.0)
    nc.vector.memset(zero_f, 0.0)
    nc.vector.tensor_copy(
        cx.vnat_w[:, :, :, HD : HD + 2],
        ones_f[:, None, None, :].to_broadcast([P, NT, NJ, 2]),
    )
    nc.vector.tensor_copy(
        cx.vnat_w[:, :, :, HD + 2 : VB_OFS],
        zero_f[:, None, None, :].to_broadcast([P, NT, NJ, VB_OFS - HD - 2]),
    )
    nc.vector.tensor_copy(cx.r_ab, zero_f[0:65, None, :].to_broadcast([65, 2, QC]))
    nc.vector.tensor_copy(cx.sel65, zero_f[0:65, :].to_broadcast([65, P]))
    nc.vector.tensor_copy(cx.sel65[64:65, 0:HD], ones_f[0:1, :].to_broadcast([1, HD]))
    nc.vector.tensor_copy(cx.sel65[32:33, HD:P], ones_f[0:1, :].to_broadcast([1, HD]))
    if not USE_XBAR:
        from concourse.masks import make_identity
        cx.ident = persist.tile([P, P], f16, name="ident")
        make_identity(nc, cx.ident)
        cx.x_s = persist.tile([P, NT, C], f16, name="x_s")
    return cx


def _iter_body(cx, x_d, qkvw_d, qkvb_d, projw_d, projb_d, phases="all"):
    """One full attention iteration: input DMAs + compute + output DMAs."""
    nc = cx.nc
    dbg = cx.dbg
    out_r = cx.out_r
    xT_s, wq_s, wp_s, qkT_s = cx.xT_s, cx.wq_s, cx.wp_s, cx.qkT_s
    vnat_s, concatT_s = cx.vnat_s, cx.concatT_s

    # ---- DMAs (HWDGE queues: SP + ACT) ----
    x_r = x_d.rearrange("(h n) c -> h n c", h=2)
    wq_src = qkvw_d.rearrange("(c p) n -> p c n", p=P)
    wp_src = projw_d.rearrange("(c p) n -> p c n", p=P)
    if USE_XBAR:
        nc.sync.dma_start_transpose(xT_s[:, :, 0 : N // 2], x_r[0])
        nc.sync.dma_start_transpose(xT_s[:, :, N // 2 : N], x_r[1])
    else:
        x_r2 = x_d.rearrange("(t p) c -> t p c", p=P)
        for t in range(NT):
            nc.sync.dma_start(cx.x_s[:, t], x_r2[t])
        for t in range(NT):
            for c in range(CK):
                pt = cx.fp.tile([P, 2, QC], f16, tag="fp", name=f"pt{t}_{c}")
                nc.tensor.transpose(
                    pt[:, 0, 0:P], cx.x_s[:, t, c * P : (c + 1) * P], cx.ident
                )
                nc.vector.tensor_copy(xT_s[:, c, t * P : (t + 1) * P], pt[:, 0, 0:P])
    for c in range(CK):
        nc.scalar.dma_start(wq_s[:, c, : 2 * C], wq_src[:, c, : 2 * C])
    nc.sync.dma_start(cx.qkvb_qk, qkvb_d[: 2 * C].rearrange("(m p) -> p m", p=P))
    for c in range(CK):  # v columns
        [nc.sync, nc.scalar][c % 2].dma_start(
            wq_s[:, c, 2 * C :], wq_src[:, c, 2 * C :]
        )
    nc.sync.dma_start(
        cx.vb_bc, qkvb_d[2 * C :].rearrange("(h j) -> h j", j=HD).partition_broadcast(P)
    )
    nc.scalar.dma_start(cx.pb_bc, projb_d.partition_broadcast(P))
    for c in range(CK):
        [nc.sync, nc.scalar][c % 2].dma_start(wp_s[:, c], wp_src[:, c])

    if phases == "feat":
        for m in range(2 * CK):
            _qk_feat2(cx, m)
        for t in range(NT):
            _v_feat(cx, t)
        for t in range(NT):
            nc.sync.dma_start(
                out_r[t], qkT_s[:, 0:CK, t * P : (t + 1) * P].bitcast(f16)
            )
        return

    # filler queue: one group per kt slot.  v(t) completes one full pair
    # before AV first touches vnat[:, t]; qk chunks for pair j+1 drain
    # during pair j.
    fillers = [lambda t=t: _v_feat(cx, t) for t in range(NT)]
    for jn in range(2, NJ):
        fillers.append(lambda m=CK + jn: _qk_feat2(cx, m))
        fillers.append(lambda m=jn: _qk_feat2(cx, m))
    fillers = fillers[::-1]  # pop from the end

    # head: features for pairs 0 and 1
    _qk_feat2(cx, CK + 0)
    _qk_feat2(cx, 0)
    _qk_feat2(cx, CK + 1)
    _qk_feat2(cx, 1)

    # pair stream: scores(j) interleaved with AV-A(j-1); AV-B(j-1) and the
    # split normalizations run at the pair boundary (2-bank accumulator)
    def _av_block(jp, exp4p):
        psA = cx.avp.tile([P, 2, QC], f32, tag="av", name=f"psA{jp}")
        for kt in range(NT):
            _av_kt(cx, jp, 0, kt, exp4p, psA)
        _norm_half(cx, jp, 0, psA)
        psB = cx.avp.tile([P, 2, QC], f32, tag="av", name=f"psB{jp}")
        for kt in range(NT):
            _av_kt(cx, jp, 1, kt, exp4p, psB)
        _norm_half(cx, jp, 1, psB)

    prev = None          # (j-1, exp4)
    for j in range(NJ):
        exp4 = cx.exps.tile([P, NT, 4, QC], f16, tag="exp", name=f"exp4_{j}")
        for kt in range(NT):
            _score_kt(cx, j, kt, exp4)
            if fillers:
                fillers.pop()()
            if prev is not None and kt == 3:
                # halfway filler: previous pair's A-head AV stream
                psA = cx.avp.tile([P, 2, QC], f32, tag="av", name=f"psA{prev[0]}")
                for k2 in range(NT):
                    _av_kt(cx, prev[0], 0, k2, prev[1], psA)
                _norm_half(cx, prev[0], 0, psA)
        if prev is not None:
            psB = cx.avp.tile([P, 2, QC], f32, tag="av", name=f"psB{prev[0]}")
            for k2 in range(NT):
                _av_kt(cx, prev[0], 1, k2, prev[1], psB)
            _norm_half(cx, prev[0], 1, psB)
        if dbg is not None and j == 0:
            nc.sync.dma_start(dbg["expq0"], exp4[:, :, 0:2])
        prev = (j, exp4)

    # tail: AV + norm for the last pair, then projection
    _av_block(prev[0], prev[1])

    if dbg is not None:
        nc.sync.dma_start(dbg["xT"], xT_s)
        nc.sync.dma_start(dbg["qkT"], qkT_s)
        nc.sync.dma_start(dbg["vnat"], vnat_s)
        nc.sync.dma_start(dbg["concatT"], concatT_s)

    if phases == "attn":
        for t in range(NT):
            nc.sync.dma_start(
                out_r[t],
                concatT_s[:, :, t * P : (t + 1) * P].rearrange(
                    "p c n -> p (c n)"
                )[:, 0:C],
            )
        return
    for t in range(NT):
        _proj_tile(cx, t)


def build(mode=MODE, repeat=1, debug_dumps=False, phases="all"):
    nc = bacc.Bacc(
        "TRN2",
        target_bir_lowering=False,
        debug=False,
        enable_asserts=False,
        num_devices=B,
    )
    x_d = nc.dram_tensor("x", [N, C], f16, kind="ExternalInput").ap()
    qkvw_d = nc.dram_tensor("qkv_w", [C, C3], f16, kind="ExternalInput").ap()
    qkvb_d = nc.dram_tensor("qkv_b", [C3], f32, kind="ExternalInput").ap()
    projw_d = nc.dram_tensor("proj_w", [C, C], f16, kind="ExternalInput").ap()
    projb_d = nc.dram_tensor("proj_b", [C], f32, kind="ExternalInput").ap()
    out_d = nc.dram_tensor("out", [N, C], f16, kind="ExternalOutput").ap()

    dbg = None
    if debug_dumps:
        dbg = {
            "xT": nc.dram_tensor("dbg_xT", [P, CK, N], f16, kind="ExternalOutput").ap(),
            "qkT": nc.dram_tensor("dbg_qkT", [P, 2 * CK, N], f16, kind="ExternalOutput").ap(),
            "vnat": nc.dram_tensor("dbg_vnat", [P, NT, NJ * PW], f16, kind="ExternalOutput").ap(),
            "expq0": nc.dram_tensor("dbg_expq0", [P, NT, 2, QC], f16, kind="ExternalOutput").ap(),
            "concatT": nc.dram_tensor("dbg_concatT", [P, CK, N], f16, kind="ExternalOutput").ap(),
        }

    with tile.TileContext(nc) as tc:
        with (
            tc.tile_pool(name="persist", bufs=1) as persist,
            tc.tile_pool(name="exps", bufs=2) as exps,
            tc.tile_pool(name="outs", bufs=2) as outs,
            tc.tile_pool(name="rbcp", bufs=2) as rbcp,
            tc.tile_pool(name="sc", bufs=1, space="PSUM") as sc,
            tc.tile_pool(name="fp", bufs=1, space="PSUM") as fp,
            tc.tile_pool(name="avp", bufs=1, space="PSUM") as avp,
        ):
            cx = _setup(tc, persist, exps, outs, rbcp, sc, fp, avp, out_d, dbg)
            if repeat == 1:
                _iter_body(cx, x_d, qkvw_d, qkvb_d, projw_d, projb_d, phases)
            else:
                with tc.For_i(
                    0, repeat, 1,
                    hint_engines=(mybir.EngineType.PE, mybir.EngineType.DVE),
                ):
                    _iter_body(cx, x_d, qkvw_d, qkvb_d, projw_d, projb_d, phases)
    nc.compile()
    return nc


_NC_CACHE = {}


def _get_nc(mode, repeat=1):
    key = (mode, repeat)
    if key not in _NC_CACHE:
        _NC_CACHE[key] = build(mode, repeat)
    return _NC_CACHE[key]


def _make_in_maps(inputs):
    x = np.asarray(inputs["x"]).astype(np.float16)
    qkv_w = np.asarray(inputs["qkv_w"]).astype(np.float16)
    qkv_b = np.asarray(inputs["qkv_b"], dtype=np.float32)
    proj_w = np.asarray(inputs["proj_w"]).astype(np.float16)
    proj_b = np.asarray(inputs["proj_b"], dtype=np.float32)
    return [
        {
            "x": np.ascontiguousarray(x[b]),
            "qkv_w": qkv_w,
            "qkv_b": qkv_b,
            "proj_w": proj_w,
            "proj_b": proj_b,
        }
        for b in range(B)
    ]


def kernel(x, qkv_w, qkv_b, proj_w, proj_b):
    nc = _get_nc(MODE, 1)
    in_maps = _make_in_maps(
        {"x": x, "qkv_w": qkv_w, "qkv_b": qkv_b, "proj_w": proj_w, "proj_b": proj_b}
    )
    res = run_bass_kernel_spmd(nc, in_maps, core_ids=list(range(B)))
    return np.stack(
        [res.results[b]["out"].astype(np.float32) for b in range(B)]
    )
